# revision 17
# baseline (speedup 1.0000x reference)
"""GAT message-passing kernel for Trainium2, 8 NeuronCores — v2.

Problem (hardcoded): B=4, N=1024, H=F=O=G=128, E=16.
  features = concat([n_features, hidden], -1)            [B,N,256]
  values   = features @ W_m + b_m                        [B,N,128]
  logits   = att1 + att2^T + (e_features@w_ae) + att_g   [B,N,N]
  coefs    = softmax(leaky_relu(logits) + (adj-1)*1e9)
  out      = coefs @ values + features @ W_skip + b_skip

Sharding: 8 cores = (batch b = core//2) x (row half = core%2): each core
owns 512 query rows of one batch; keys unsharded. No collectives.

v2 design (~80 us v1 -> target ~45 us): memory-bound on the e_features
stream, so halve the bytes and keep every engine off the DMA critical
path.
  - ef is quantized host-side to fp8 e4m3 with ERROR-FEEDBACK across
    the E dim (descending |w_ae|): each channel's quantization residual
    is carried into the next channel, so the on-chip dot product
    sum_e ef_e*w_e lands at bf16-level accuracy (2.7e-3) at half the
    HBM traffic (8 MiB/core).
  - TRANSPOSED attention: logits are built as lps[key, row] per
    128-key chunk. Softmax normalization moves to a ones-matmul
    (PSUM-accumulated across chunks) and A@V consumes the masked
    coefs^T tile directly as the PE moving operand with V stationary —
    no PE transposes, no PSUM->SBUF copies on ACT.
  - The E-contraction runs in DoubleRow fp8 (2 elems/lane/cycle):
    stationary Wd[(km2,es),j,km'] block-diagonal over 64 keys with 4
    channels (es,j) per s-group; out partitions 0..63 (DR requires full
    128-col footprint), so the pipeline works on two [64,512] PSUM
    tiles per chunk.
  - exp(leaky_relu(x)) = max(exp(x), exp(.01x)) on ACT with att2+att_g
    +biases on the per-partition bias port; att1 rides a rank-1 matmul
    into PSUM. adj mask multiplies post-exp on DVE.
  - Output is produced transposed [O, rows] and un-transposed on host.
"""

import os
import numpy as np

B, N, H, F, E, G, O = 4, 1024, 128, 128, 16, 128, 128
DIN = F + H
NCORES = 8
ROWS = N // 2          # query rows per core
KC = N // 128          # key chunks of 128
KG = 2                 # 64-key groups per chunk
NS = 4                 # s-groups (4 channels (es,j) each)
NPAIR = KC // 2        # ef DMA tiles (2 chunks each)

_cache = {}


def _build():
    from contextlib import ExitStack
    import concourse.bacc as bacc
    import concourse.tile as tile
    import concourse.mybir as mybir

    fp32 = mybir.dt.float32
    bf16 = mybir.dt.bfloat16
    f8 = mybir.dt.float8e4
    AF = mybir.ActivationFunctionType
    DR = mybir.MatmulPerfMode.DoubleRow

    nc = bacc.Bacc("TRN2", target_bir_lowering=False, debug=False,
                   num_devices=NCORES)

    # T[p=(km2*2+es), kc, kg, s, j, r] fp8 codes of ef
    T_in = nc.dram_tensor("T", [128, KC, KG, NS, 2, ROWS], f8,
                          kind="ExternalInput")
    Wd_in = nc.dram_tensor("Wd", [128, NS, 2, 64], f8, kind="ExternalInput")
    adjT_in = nc.dram_tensor("adjT", [64, KC, KG, ROWS], bf16,
                             kind="ExternalInput")
    fTk_in = nc.dram_tensor("fTk", [128, 2, N], bf16, kind="ExternalInput")
    fTr_in = nc.dram_tensor("fTr", [128, 2, ROWS], bf16, kind="ExternalInput")
    Wm_in = nc.dram_tensor("Wm", [128, 2, O], bf16, kind="ExternalInput")
    Wsk_in = nc.dram_tensor("Wsk", [128, 2, O], bf16, kind="ExternalInput")
    bsk_in = nc.dram_tensor("bsk", [1, O], bf16, kind="ExternalInput")
    w12_in = nc.dram_tensor("w12", [128, 4], bf16, kind="ExternalInput")
    g_in = nc.dram_tensor("g", [G, 1], bf16, kind="ExternalInput")
    wag_in = nc.dram_tensor("wag", [G, 1], bf16, kind="ExternalInput")
    bs_in = nc.dram_tensor("bs", [1, 1], fp32, kind="ExternalInput")
    out_t = nc.dram_tensor("out", [O, ROWS], fp32, kind="ExternalOutput")

    with tile.TileContext(nc) as tc:
        with ExitStack() as ctx:
            singles = ctx.enter_context(tc.tile_pool(name="singles", bufs=1))
            efp = ctx.enter_context(tc.tile_pool(name="efp", bufs=8))
            work = ctx.enter_context(tc.tile_pool(name="work", bufs=3))
            psL = ctx.enter_context(tc.tile_pool(name="psL", bufs=2,
                                                 space="PSUM"))
            psO = ctx.enter_context(tc.tile_pool(name="psO", bufs=1,
                                                 space="PSUM"))
            psS = ctx.enter_context(tc.tile_pool(name="psS", bufs=1,
                                                 space="PSUM"))
            psR = ctx.enter_context(tc.tile_pool(name="psR", bufs=2,
                                                 space="PSUM"))

            # ---- prime the ef stream (sync HWDGE queue) ----------------
            ef_tiles = []
            t0 = efp.tile([128, KG, NS, 2, ROWS], f8, tag="ef")
            nc.sync.dma_start(out=t0[:, 0], in_=T_in[:, 0, 0])
            nc.sync.dma_start(out=t0[:, 1], in_=T_in[:, 0, 1])
            ef_tiles.append(t0)
            for p in range(1, KC):
                t = efp.tile([128, KG, NS, 2, ROWS], f8, tag="ef")
                nc.sync.dma_start(out=t, in_=T_in[:, p])
                ef_tiles.append(t)

            # ---- constants (scalar HWDGE queue) ------------------------
            Wd_sb = singles.tile([128, NS, 2, 64], f8)
            nc.scalar.dma_start(out=Wd_sb, in_=Wd_in.ap())
            fTk = singles.tile([128, 2, N], bf16)
            nc.scalar.dma_start(out=fTk, in_=fTk_in.ap())
            fTr = singles.tile([128, 2, ROWS], bf16)
            nc.scalar.dma_start(out=fTr, in_=fTr_in.ap())
            Wm_sb = singles.tile([128, 2, O], bf16)
            nc.scalar.dma_start(out=Wm_sb, in_=Wm_in.ap())
            Wsk_sb = singles.tile([128, 2, O], bf16)
            nc.scalar.dma_start(out=Wsk_sb, in_=Wsk_in.ap())
            bsk_sb = singles.tile([1, O], bf16)
            nc.scalar.dma_start(out=bsk_sb, in_=bsk_in.ap())
            w12_sb = singles.tile([128, 4], bf16)
            nc.scalar.dma_start(out=w12_sb, in_=w12_in.ap())
            g_sb = singles.tile([G, 1], bf16)
            nc.scalar.dma_start(out=g_sb, in_=g_in.ap())
            wag_sb = singles.tile([G, 1], bf16)
            nc.scalar.dma_start(out=wag_sb, in_=wag_in.ap())
            bs_sb = singles.tile([1, 1], fp32)
            nc.scalar.dma_start(out=bs_sb, in_=bs_in.ap())
            adjT = singles.tile([64, KC, KG, ROWS], bf16)
            nc.scalar.dma_start(out=adjT, in_=adjT_in.ap())

            ones_bf = singles.tile([1, 128], bf16)
            nc.vector.memset(ones_bf, 1.0)
            ones512 = singles.tile([1, ROWS], bf16)
            nc.vector.memset(ones512, 1.0)
            ones64c = singles.tile([64, 1], bf16)
            nc.vector.memset(ones64c, 1.0)
            ones_f32 = singles.tile([1, 128], fp32)
            nc.vector.memset(ones_f32, 1.0)
            w0 = singles.tile([128, 128], bf16)
            nc.vector.memset(w0, 0.0)

            # PE warmup spin: sustained activity flips HAM to 8/8 early
            wups = psR.tile([128, 128], fp32, tag="ret")
            for i in range(40):
                nc.tensor.matmul(wups, w0, w0, start=True, stop=True,
                                 tile_position=(0, 0), skip_group_check=True)

            # ---- phase 0: V, att2, att1, skipT, att_g ------------------
            # att_g + sum(biases): sc = g@wag + bs -> bcast [64,1]
            gps = psR.tile([1, 1], fp32, tag="ret")
            nc.tensor.matmul(gps, g_sb, wag_sb, start=True, stop=True)
            sc1 = singles.tile([1, 1], fp32)
            nc.vector.tensor_copy(sc1, gps)
            nc.vector.tensor_scalar_add(sc1, sc1, bs_sb)


            # att1 over this core's rows, + (att_g + biases): [1, ROWS]
            a1ps = psR.tile([1, ROWS], fp32, tag="ret")
            nc.tensor.matmul(a1ps, w12_sb[:, 0:1], fTr[:, 0, :],
                             start=True, stop=False)
            nc.tensor.matmul(a1ps, w12_sb[:, 1:2], fTr[:, 1, :],
                             start=False, stop=True)
            a1f = singles.tile([1, ROWS], fp32)
            nc.vector.tensor_scalar_add(a1f, a1ps, sc1)
            att1r = singles.tile([1, ROWS], bf16)
            nc.vector.tensor_copy(att1r, a1f)

            # skip^T[o, r] = W_skip^T @ features_r + b_skip
            skp = psR.tile([128, ROWS], fp32, tag="ret")
            nc.tensor.matmul(skp, Wsk_sb[:, 0, :], fTr[:, 0, :],
                             start=True, stop=False)
            nc.tensor.matmul(skp, Wsk_sb[:, 1, :], fTr[:, 1, :],
                             start=False, stop=False)
            nc.tensor.matmul(skp, bsk_sb, ones512, start=False, stop=True,
                             skip_group_check=True)
            skipT = singles.tile([128, ROWS], fp32)
            nc.vector.tensor_copy(skipT, skp)

            # ---- main loop over key chunks (software-pipelined) --------
            outT_ps = psO.tile([128, ROWS], fp32, tag="o")
            s_ps = psS.tile([1, ROWS], fp32, tag="s")
            V_sb = singles.tile([64, KC * KG, O], bf16)
            att2b = singles.tile([64, KC * KG], fp32)
            cT_tiles = {}

            def contract(kc):
                Ekc = ef_tiles[kc]          # [128, KG, NS, 2, ROWS]
                lps = []
                for kg in range(KG):
                    lp = psL.tile([64, ROWS], fp32, tag=f"l{kg}")
                    for s in range(NS):
                        nc.tensor.matmul(lp, Wd_sb[:, s], Ekc[:, kg, s],
                                         start=(s == 0), stop=False,
                                         perf_mode=DR, tile_position=(0, 0))
                    nc.tensor.matmul(lp, ones_bf[:, 0:64], att1r,
                                     start=False, stop=True,
                                     skip_group_check=True)
                    lps.append(lp)
                return lps

            def vmm(kc):
                # JIT V[key64, O] + att2[key64] for this chunk's two halves
                for kg in range(KG):
                    idx = kc * KG + kg
                    ks = slice(kc * 128 + kg * 64, kc * 128 + kg * 64 + 64)
                    vps = psR.tile([64, O], fp32, tag="ret")
                    nc.tensor.matmul(vps, fTk[:, 0, ks], Wm_sb[:, 0, :],
                                     start=True, stop=False)
                    nc.tensor.matmul(vps, fTk[:, 1, ks], Wm_sb[:, 1, :],
                                     start=False, stop=True)
                    nc.vector.tensor_copy(V_sb[:, idx, :], vps)
                    aps = psR.tile([64, 1], fp32, tag="ret")
                    nc.tensor.matmul(aps, fTk[:, 0, ks], w12_sb[:, 2:3],
                                     start=True, stop=False)
                    nc.tensor.matmul(aps, fTk[:, 1, ks], w12_sb[:, 3:4],
                                     start=False, stop=True)
                    nc.vector.tensor_copy(att2b[:, idx:idx + 1], aps)

            def softmax_mask(kc, lps):
                lrl = work.tile([64, KG, ROWS], fp32, tag="lrl")
                ex = work.tile([64, KG, ROWS], bf16, tag="ex")
                for kg in range(KG):
                    idx = kc * KG + kg
                    nc.scalar.activation(lrl[:, kg, :], lps[kg], AF.Lrelu,
                                         bias=att2b[:, idx:idx + 1],
                                         alpha=0.01)
                    nc.scalar.activation(ex[:, kg, :], lrl[:, kg, :], AF.Exp)
                cT = work.tile([64, KG, ROWS], bf16, tag="cT")
                nc.vector.tensor_mul(cT, ex, adjT[:, kc])
                cT_tiles[kc] = cT

            def av(kc):
                cT = cT_tiles.pop(kc)
                for kg in range(KG):
                    idx = kc * KG + kg
                    first = (kc == 0 and kg == 0)
                    last = (kc == KC - 1 and kg == KG - 1)
                    nc.tensor.matmul(s_ps, ones64c, cT[:, kg, :],
                                     start=first, stop=last,
                                     skip_group_check=True)
                    nc.tensor.matmul(outT_ps, V_sb[:, idx, :], cT[:, kg, :],
                                     start=first, stop=last,
                                     skip_group_check=True)

            for kc in range(KC):
                if kc >= 2:
                    av(kc - 2)
                vmm(kc)
                lps = contract(kc)
                softmax_mask(kc, lps)
            av(KC - 2)
            av(KC - 1)

            # ---- normalize + add skip + store --------------------------
            s_sb = singles.tile([1, ROWS], fp32)
            nc.vector.tensor_copy(s_sb, s_ps)
            rec = singles.tile([1, ROWS], fp32)
            nc.vector.reciprocal(rec, s_sb)
            rb_ps = psR.tile([128, ROWS], fp32, tag="ret")
            nc.tensor.matmul(rb_ps, ones_f32, rec, start=True, stop=True)
            rb_sb = singles.tile([128, ROWS], fp32)
            nc.scalar.copy(out=rb_sb, in_=rb_ps)
            tmp = singles.tile([128, ROWS], fp32)
            nc.vector.tensor_mul(tmp, outT_ps, rb_sb)
            out_sb = singles.tile([128, ROWS], fp32)
            nc.vector.tensor_add(out_sb, tmp, skipT)
            nc.sync.dma_start(out=out_t.ap(), in_=out_sb)

    nc.compile()
    return nc


def _get_nc():
    if "nc" not in _cache:
        _cache["nc"] = _build()
    return _cache["nc"]


def _quantize_ef_feedback(e_features, w_ae):
    """fp8 e4m3 codes for ef with error feedback across E (desc |w|).

    Returns (codes [B,N,N,E] e4m3, w_hat [E] f32)."""
    import ml_dtypes
    f8 = ml_dtypes.float8_e4m3
    f32 = np.float32
    w = np.asarray(w_ae, f32).reshape(E)
    wh = w.astype(f8).astype(f32)
    order = np.argsort(-np.abs(w))
    ef = np.asarray(e_features, f32)
    codes = np.empty(ef.shape, dtype=f8)
    carry = np.zeros(ef.shape[:-1], f32)
    for e in order:
        x = (ef[..., e] * w[e] + carry) / wh[e]
        qc = x.astype(f8)
        codes[..., e] = qc
        carry = x * wh[e] - qc.astype(f32) * wh[e]
    return codes, wh


def _in_maps(hidden, n_features, e_features, g_features, adj,
             W_m, b_m, W_skip, b_skip, w_a1, b_a1, w_a2, b_a2,
             w_ae, b_ae, w_ag, b_ag):
    import ml_dtypes
    bf16 = ml_dtypes.bfloat16
    f8 = ml_dtypes.float8_e4m3
    f32 = np.float32
    asb = lambda x: np.ascontiguousarray(np.asarray(x).astype(bf16))
    bsum = (np.float32(np.asarray(b_a1).reshape(())) +
            np.float32(np.asarray(b_a2).reshape(())) +
            np.float32(np.asarray(b_ae).reshape(())) +
            np.float32(np.asarray(b_ag).reshape(())))

    codes, wh = _quantize_ef_feedback(e_features, w_ae)

    # Wd[(km2*2+es), s, j, km'] = (km2==km') * w_hat[s*4+es*2+j]
    Wd = np.zeros((64, 2, NS, 2, 64), f32)      # [km2, es, s, j, km']
    for es in range(2):
        for s in range(NS):
            for j in range(2):
                np.fill_diagonal(Wd[:, es, s, j, :], wh[s * 4 + es * 2 + j])
    Wd = Wd.reshape(128, NS, 2, 64).astype(f8)

    w12 = np.stack([np.asarray(w_a1, f32).reshape(2, 128)[0],
                    np.asarray(w_a1, f32).reshape(2, 128)[1],
                    np.asarray(w_a2, f32).reshape(2, 128)[0],
                    np.asarray(w_a2, f32).reshape(2, 128)[1]], axis=1)
    shared = {
        "Wd": np.ascontiguousarray(Wd),
        "Wm": asb(W_m).reshape(2, 128, O).transpose(1, 0, 2),
        "Wsk": asb(W_skip).reshape(2, 128, O).transpose(1, 0, 2),
        # b_m folded into the skip bias: sum_k c_k (V+b_m) / s = ... + b_m
        "bsk": asb(np.asarray(b_skip, f32) + np.asarray(b_m, f32)).reshape(1, O),
        "w12": asb(w12), "wag": asb(w_ag),
        "bs": np.array([[bsum]], dtype=f32),
    }
    shared["Wm"] = np.ascontiguousarray(shared["Wm"])
    shared["Wsk"] = np.ascontiguousarray(shared["Wsk"])
    maps = []
    for c in range(NCORES):
        b, h = c // 2, c % 2
        rows = slice(h * ROWS, (h + 1) * ROWS)
        m = dict(shared)
        # T[(km2*2+es), kc, kg, s, j, r] = codes[r, kc*128+kg*64+km2, s*4+es*2+j]
        Q = codes[b, rows]                              # [512,1024,16] f8
        Q = Q.reshape(ROWS, KC, KG, 64, NS, 2, 2)       # r,kc,kg,km2,s,es,j
        Q = Q.transpose(3, 5, 1, 2, 4, 6, 0)            # km2,es,kc,kg,s,j,r
        m["T"] = np.ascontiguousarray(Q.reshape(128, KC, KG, NS, 2, ROWS))
        A = np.asarray(adj[b], f32)[rows]               # [512,1024]
        AT = A.T.reshape(KC, KG, 64, ROWS).transpose(2, 0, 1, 3)
        m["adjT"] = np.ascontiguousarray(AT.astype(bf16))
        fk = np.stack([np.asarray(n_features[b], f32).T,
                       np.asarray(hidden[b], f32).T])   # [2,128,1024]
        m["fTk"] = np.ascontiguousarray(
            fk.transpose(1, 0, 2).astype(bf16))
        fr = np.stack([np.asarray(n_features[b], f32)[rows].T,
                       np.asarray(hidden[b], f32)[rows].T])
        m["fTr"] = np.ascontiguousarray(
            fr.transpose(1, 0, 2).astype(bf16))
        m["g"] = asb(g_features[b]).reshape(G, 1)
        maps.append(m)
    return maps


def kernel(hidden, n_features, e_features, g_features, adj,
           W_m, b_m, W_skip, b_skip, w_a1, b_a1, w_a2, b_a2,
           w_ae, b_ae, w_ag, b_ag):
    from concourse import bass_utils
    nc = _get_nc()
    maps = _in_maps(hidden, n_features, e_features, g_features, adj,
                    W_m, b_m, W_skip, b_skip, w_a1, b_a1, w_a2, b_a2,
                    w_ae, b_ae, w_ag, b_ag)
    res = bass_utils.run_bass_kernel_spmd(nc, maps, core_ids=list(range(NCORES)))
    out = np.empty((B, N, O), np.float32)
    for c in range(NCORES):
        b, h = c // 2, c % 2
        out[b, h * ROWS:(h + 1) * ROWS] = res.results[c]["out"].T
    return out
